# revision 1
# baseline (speedup 1.0000x reference)
"""Trainium2 Bass kernel for quantized-MLP-with-LoRA (nn_MixedSparseTraditionalMLP).

Strategy: data-parallel over the 8192 tokens across 8 NeuronCores (1024 tokens
per core). Each core holds the full weights, dequantizes the 4-bit codes to
fp16 on-chip (DVE cast+offset, then one broadcast-scale multiply), transposes
operands into contraction-major layout with the DMA xbar (SBUF->SBUF), and runs
both projections as fp16 matmuls with fp32 PSUM accumulation. LoRA terms and
the down-projection bias are folded into the same PSUM accumulation groups as
extra low-rank matmul steps; relu + up-bias are applied by the scalar engine on
the PSUM->SBUF copy. x2 (the hidden activation) round-trips through DRAM in
transposed layout so the down projection streams it as the stationary operand.
No collectives: the host just concatenates the 8 per-core token slices.
"""
import sys

if "/opt/trn_rl_repo" not in sys.path:
    sys.path.insert(0, "/opt/trn_rl_repo")

import numpy as np

import concourse.bass as bass
import concourse.mybir as mybir
import concourse.tile as tile
from concourse import bacc
from concourse.bass import ts, ds
from concourse.bass_utils import run_bass_kernel_spmd

F16 = mybir.dt.float16
F32 = mybir.dt.float32
I32 = mybir.dt.int32

NCORES = 8
T = 1024          # tokens per core
D = 2048
H = 8192
R = 16
P = 128
KD = D // P       # 16 k-subtiles for the up contraction
KH = H // P       # 64 k-subtiles for the down contraction
NT = T // 512     # 2 moving-operand tiles of 512 tokens
DM = 4            # down-projection d tiles of 512

TRACE = False
LAST_RESULTS = None


def _build():
    nc = bacc.Bacc("TRN2", target_bir_lowering=False, debug=False,
                   enable_asserts=False, num_devices=NCORES)

    x1c = nc.dram_tensor("x1c", [T, D], F32, kind="ExternalInput").ap()
    wupq = nc.dram_tensor("wupq", [H, D], I32, kind="ExternalInput").ap()
    sup = nc.dram_tensor("sup", [H, D // 64], F32, kind="ExternalInput").ap()
    bup = nc.dram_tensor("bup", [H], F32, kind="ExternalInput").ap()
    a1 = nc.dram_tensor("a1", [D, R], F32, kind="ExternalInput").ap()
    b1 = nc.dram_tensor("b1", [R, H], F32, kind="ExternalInput").ap()
    wdnq = nc.dram_tensor("wdnq", [D, H], I32, kind="ExternalInput").ap()
    sdn = nc.dram_tensor("sdn", [D, H // 64], F32, kind="ExternalInput").ap()
    bdn = nc.dram_tensor("bdn", [D], F32, kind="ExternalInput").ap()
    a2 = nc.dram_tensor("a2", [H, R], F32, kind="ExternalInput").ap()
    b2 = nc.dram_tensor("b2", [R, D], F32, kind="ExternalInput").ap()
    y2c = nc.dram_tensor("y2c", [T, D], F32, kind="ExternalOutput").ap()

    with tile.TileContext(nc) as tc:
        with tc.tile_pool(name="const", bufs=1) as cp, \
             tc.tile_pool(name="dram", bufs=1, space="DRAM") as dp, \
             tc.tile_pool(name="psum", bufs=4, space="PSUM") as pp, \
             tc.tile_pool(name="psum_vt", bufs=1, space="PSUM") as pvt:

            x2td = dp.tile([KH, P, T], F16)     # transposed hidden activation

            # constants that live through both phases
            sdnf = cp.tile([P, D // P, H // 64], F16, tag="sdnf")
            a2f = cp.tile([P, KH, R], F16, tag="a2f")
            b2p = cp.tile([R + 1, D], F16, tag="b2p")
            v1t = cp.tile([R + 1, T], F16, tag="v1t")
            # row R must read 1.0 (folds b_down into the lora matmul); rows
            # 0..R-1 are overwritten with vT after the up phase
            nc.any.memset(v1t[:], 1.0)

            vt_ps = [pvt.tile([R, 512], F32, tag=f"vt{i}", name=f"vt{i}")
                     for i in range(NT)]

            with tc.tile_pool(name="upc", bufs=1) as up, \
                 tc.tile_pool(name="stage", bufs=3) as sp, \
                 tc.tile_pool(name="wup", bufs=3) as wp, \
                 tc.tile_pool(name="x2s", bufs=3) as xp:

                x1t = up.tile([P, KD, T], F16, tag="x1t")
                supf = up.tile([P, H // P, D // 64], F16, tag="supf")
                a1f = up.tile([P, KD, R], F16, tag="a1f")
                b1f = up.tile([R, H], F16, tag="b1f")
                bupsb = up.tile([P, KH], F32, tag="bupsb")
                utf = up.tile([R, T], F16, tag="utf")

                # ---- prep: scales, lora mats, biases (via f32 staging tiles) ----
                stx = sp.tile([P, D], F32, tag="st32")
                v = stx[:].rearrange("p (o b) -> p o b", b=H // 64)  # [128,16,128]
                nc.sync.dma_start(v, sdn.rearrange("(o p) b -> p o b", p=P))
                nc.vector.tensor_copy(sdnf[:], v)

                stx = sp.tile([P, D], F32, tag="st32")
                v = stx[:, : KH * R].rearrange("p (o r) -> p o r", r=R)
                nc.sync.dma_start(v, a2.rearrange("(o p) r -> p o r", p=P))
                nc.vector.tensor_copy(a2f[:], v)

                stx = sp.tile([P, D], F32, tag="st32")
                nc.sync.dma_start(stx[:R, :], b2)
                nc.sync.dma_start(stx[R:R + 1, :], bdn[None, :])
                nc.vector.tensor_copy(b2p[:], stx[:R + 1, :])

                stx = sp.tile([P, D], F32, tag="st32")
                v = stx[:].rearrange("p (o b) -> p o b", b=D // 64)  # [128,64,32]
                nc.sync.dma_start(v, sup.rearrange("(o p) b -> p o b", p=P))
                nc.vector.tensor_copy(supf[:], v)

                stx = sp.tile([P, D], F32, tag="st32")
                v = stx[:, : KD * R].rearrange("p (o r) -> p o r", r=R)
                nc.sync.dma_start(v, a1.rearrange("(o p) r -> p o r", p=P))
                nc.vector.tensor_copy(a1f[:], v)

                for c in range(4):
                    stx = sp.tile([P, D], F32, tag="st32")
                    nc.sync.dma_start(stx[:R, :], b1[:, ts(c, D)])
                    nc.vector.tensor_copy(b1f[:, ts(c, D)], stx[:R, :])
                nc.sync.dma_start(bupsb[:], bup.rearrange("(o p) -> p o", p=P))

                # ---- x1 -> fp16, transposed to [d_partition, d_subtile, token] ----
                for s in range(T // P):
                    stx = sp.tile([P, D], F32, tag="st32")
                    nc.sync.dma_start(stx[:], x1c[ts(s, P), :])
                    xf = sp.tile([P, D], F16, tag="xf")
                    nc.vector.tensor_copy(xf[:], stx[:])
                    nc.sync.dma_start_transpose(x1t[:, :, ts(s, P)], xf[:])

                # ---- uT = (x1 @ A1)^T : [R, T] ----
                for tt in range(NT):
                    ups = pp.tile([R, 512], F32, tag="mm")
                    for j in range(KD):
                        nc.tensor.matmul(ups[:], a1f[:, j, :], x1t[:, j, ts(tt, 512)],
                                         start=(j == 0), stop=(j == KD - 1))
                    nc.scalar.copy(utf[:, ts(tt, 512)], ups[:])

                # ---- UP: one 128-row slab of H per step ----
                for k in range(KH):
                    qst = sp.tile([P, D], I32, tag="qst")
                    nc.sync.dma_start(qst[:], wupq[ts(k, P), :])
                    qf = sp.tile([P, D], F16, tag="qf")
                    nc.vector.tensor_scalar_add(qf[:], qst[:], -7.5)
                    nc.vector.tensor_tensor(
                        qf[:].rearrange("p (b i) -> p b i", i=64),
                        qf[:].rearrange("p (b i) -> p b i", i=64),
                        supf[:, k, :, None].to_broadcast((P, D // 64, 64)),
                        mybir.AluOpType.mult)
                    wt = wp.tile([P, KD, P], F16, tag="wupt")
                    nc.sync.dma_start_transpose(wt[:], qf[:])

                    x2sl = xp.tile([P, T], F16, tag="x2sl")
                    for tt in range(NT):
                        ps = pp.tile([P, 512], F32, tag="mm")
                        for j in range(KD):
                            nc.tensor.matmul(ps[:], wt[:, j, :], x1t[:, j, ts(tt, 512)],
                                             start=(j == 0), stop=False)
                        nc.tensor.matmul(ps[:], b1f[:, ts(k, P)], utf[:, ts(tt, 512)],
                                         start=False, stop=True)
                        nc.scalar.activation(x2sl[:, ts(tt, 512)], ps[:],
                                             mybir.ActivationFunctionType.Relu,
                                             bias=bupsb[:, k:k + 1], scale=1.0)
                        nc.tensor.matmul(vt_ps[tt][:], a2f[:, k, :], x2sl[:, ts(tt, 512)],
                                         start=(k == 0), stop=(k == KH - 1),
                                         skip_group_check=True)
                    nc.sync.dma_start(x2td[k], x2sl[:])

                for tt in range(NT):
                    nc.scalar.copy(v1t[:R, ts(tt, 512)], vt_ps[tt][:])

            # ---- DOWN: stream x2^T tiles and dequantized w_down tiles ----
            with tc.tile_pool(name="wdn", bufs=2) as wd, \
                 tc.tile_pool(name="x2r", bufs=2) as xr, \
                 tc.tile_pool(name="dstage", bufs=2) as dsp, \
                 tc.tile_pool(name="yout", bufs=2) as yp:
                for m in range(DM):
                    wdt = wd.tile([P, KH, 512], F16, tag="wdt")
                    for s in range(4):           # 128-row d slabs within the 512 tile
                        d0 = 512 * m + 128 * s
                        for c in range(8):       # 1024-wide h chunks
                            qst = dsp.tile([P, 1024], I32, tag="qst")
                            nc.sync.dma_start(qst[:], wdnq[ds(d0, P), ts(c, 1024)])
                            qf = dsp.tile([P, 1024], F16, tag="qf")
                            nc.vector.tensor_scalar_add(qf[:], qst[:], -7.5)
                            nc.vector.tensor_tensor(
                                qf[:].rearrange("p (b i) -> p b i", i=64),
                                qf[:].rearrange("p (b i) -> p b i", i=64),
                                sdnf[:, 4 * m + s, ds(16 * c, 16), None].to_broadcast(
                                    (P, 16, 64)),
                                mybir.AluOpType.mult)
                            nc.sync.dma_start_transpose(
                                wdt[:, ds(8 * c, 8), ts(s, P)], qf[:])
                    for t8 in range(T // P):
                        x2r_t = xr.tile([P, KH, P], F16, tag="x2r")
                        nc.sync.dma_start(
                            x2r_t[:], x2td[:, :, ts(t8, P)].rearrange("k p t -> p k t"))
                        ps = pp.tile([P, 512], F32, tag="mm")
                        for k in range(KH):
                            nc.tensor.matmul(ps[:], x2r_t[:, k, :], wdt[:, k, :],
                                             start=(k == 0), stop=False)
                        nc.tensor.matmul(ps[:], v1t[:, ts(t8, P)], b2p[:, ts(m, 512)],
                                         start=False, stop=True)
                        yo = yp.tile([P, 512], F32, tag="yo")
                        nc.scalar.copy(yo[:], ps[:])
                        nc.sync.dma_start(y2c[ts(t8, P), ts(m, 512)], yo[:])

    nc.compile()
    return nc


_NC = None


def kernel(x1, w_up_q, w_up_scale, b_up, w_up_lora_a, w_up_lora_b,
           w_down_q, w_down_scale, b_down, w_down_lora_a, w_down_lora_b):
    global _NC, LAST_RESULTS
    if _NC is None:
        _NC = _build()

    x1 = np.ascontiguousarray(np.asarray(x1, dtype=np.float32))
    B, S, _ = x1.shape
    xf = x1.reshape(B * S, D)
    shared = {
        "wupq": np.ascontiguousarray(np.asarray(w_up_q, dtype=np.int32)),
        "sup": np.ascontiguousarray(np.asarray(w_up_scale, dtype=np.float32)),
        "bup": np.ascontiguousarray(np.asarray(b_up, dtype=np.float32)),
        "a1": np.ascontiguousarray(np.asarray(w_up_lora_a, dtype=np.float32)),
        "b1": np.ascontiguousarray(np.asarray(w_up_lora_b, dtype=np.float32)),
        "wdnq": np.ascontiguousarray(np.asarray(w_down_q, dtype=np.int32)),
        "sdn": np.ascontiguousarray(np.asarray(w_down_scale, dtype=np.float32)),
        "bdn": np.ascontiguousarray(np.asarray(b_down, dtype=np.float32)),
        "a2": np.ascontiguousarray(np.asarray(w_down_lora_a, dtype=np.float32)),
        "b2": np.ascontiguousarray(np.asarray(w_down_lora_b, dtype=np.float32)),
    }
    in_maps = [{"x1c": np.ascontiguousarray(xf[c * T:(c + 1) * T]), **shared}
               for c in range(NCORES)]

    res = run_bass_kernel_spmd(_NC, in_maps, core_ids=list(range(NCORES)),
                               trace=TRACE)
    LAST_RESULTS = res
    out = np.concatenate([res.results[c]["y2c"] for c in range(NCORES)], axis=0)
    return out.reshape(B, S, D)



# revision 2
# speedup vs baseline: 1.0139x; 1.0139x over previous
"""Trainium2 Bass kernel for quantized-MLP-with-LoRA (nn_MixedSparseTraditionalMLP).

Strategy: data-parallel over the 8192 tokens across 8 NeuronCores (1024 tokens
per core). Each core holds the full weights, dequantizes the 4-bit codes to
fp16 on-chip (DVE add + broadcast-scale multiply), and transposes operands into
contraction-major layout with the TENSOR ENGINE (matmul transpose mode, 1
cycle/row for fp16 — ~53ns per 128x128 tile) instead of the DMA xbar, whose
per-element descriptors dominated the previous version (~7ns/element floor).
Both projections run as fp16 matmuls with fp32 PSUM accumulation. LoRA terms
and the down bias are folded into the same PSUM accumulation groups as extra
low-rank matmul steps; relu + up-bias are applied by the scalar engine on the
PSUM->SBUF copy. x2 round-trips through DRAM in [h_slab, token] layout (plain
contiguous DMA both ways) and is then kept fully SBUF-resident for the down
projection, so down-phase weights stream exactly once. No collectives: the
host concatenates the 8 per-core token slices.
"""
import sys

if "/opt/trn_rl_repo" not in sys.path:
    sys.path.insert(0, "/opt/trn_rl_repo")

import numpy as np

import concourse.bass as bass
import concourse.mybir as mybir
import concourse.tile as tile
from concourse import bacc
from concourse.bass import ts, ds
from concourse.bass_utils import run_bass_kernel_spmd
from concourse.masks import make_identity

F16 = mybir.dt.float16
F32 = mybir.dt.float32
I32 = mybir.dt.int32

NCORES = 8
T = 1024          # tokens per core
D = 2048
H = 8192
R = 16
P = 128
KD = D // P       # 16 k-subtiles for the up contraction
KH = H // P       # 64 k-subtiles for the down contraction
NT = T // 512     # 2 moving-operand tiles of 512 tokens

TRACE = False
LAST_RESULTS = None


def _build():
    nc = bacc.Bacc("TRN2", target_bir_lowering=False, debug=False,
                   enable_asserts=False, num_devices=NCORES)

    x1c = nc.dram_tensor("x1c", [T, D], F32, kind="ExternalInput").ap()
    wupq = nc.dram_tensor("wupq", [H, D], I32, kind="ExternalInput").ap()
    sup = nc.dram_tensor("sup", [H, D // 64], F32, kind="ExternalInput").ap()
    bup = nc.dram_tensor("bup", [H], F32, kind="ExternalInput").ap()
    a1 = nc.dram_tensor("a1", [D, R], F32, kind="ExternalInput").ap()
    b1 = nc.dram_tensor("b1", [R, H], F32, kind="ExternalInput").ap()
    wdnq = nc.dram_tensor("wdnq", [D, H], I32, kind="ExternalInput").ap()
    sdn = nc.dram_tensor("sdn", [D, H // 64], F32, kind="ExternalInput").ap()
    bdn = nc.dram_tensor("bdn", [D], F32, kind="ExternalInput").ap()
    a2 = nc.dram_tensor("a2", [H, R], F32, kind="ExternalInput").ap()
    b2 = nc.dram_tensor("b2", [R, D], F32, kind="ExternalInput").ap()
    y2c = nc.dram_tensor("y2c", [T, D], F32, kind="ExternalOutput").ap()

    with tile.TileContext(nc) as tc:
        with tc.tile_pool(name="gconst", bufs=1) as gp, \
             tc.tile_pool(name="dram", bufs=1, space="DRAM") as dp, \
             tc.tile_pool(name="psum_mm", bufs=3, space="PSUM") as pp, \
             tc.tile_pool(name="psum_tr", bufs=2, space="PSUM") as ptr:

            x2td = dp.tile([KH, P, T], F16)     # hidden activation [h_slab, t]

            # constants that live through both phases
            ident = gp.tile([P, P], F16, tag="ident")
            make_identity(nc, ident[:])
            sdnf = gp.tile([P, D // P, H // 64], F16, tag="sdnf")
            a2f = gp.tile([P, KH, R], F16, tag="a2f")
            b2p = gp.tile([R + 1, D], F16, tag="b2p")
            v1t = gp.tile([R + 1, T], F16, tag="v1t")
            # row R must read 1.0 (folds b_down into the lora matmul); rows
            # 0..R-1 are overwritten with vT after the up phase
            nc.any.memset(v1t[:], 1.0)

            with tc.tile_pool(name="upc", bufs=1) as up, \
                 tc.tile_pool(name="stage", bufs=3) as sp, \
                 tc.tile_pool(name="wup", bufs=2) as wp, \
                 tc.tile_pool(name="x2s", bufs=3) as xp, \
                 tc.tile_pool(name="psum_vt", bufs=1, space="PSUM") as pvt:

                x1t = up.tile([P, KD, T], F16, tag="x1t")
                supf = up.tile([P, H // P, D // 64], F16, tag="supf")
                a1f = up.tile([P, KD, R], F16, tag="a1f")
                b1f = up.tile([R, H], F16, tag="b1f")
                bupsb = up.tile([P, KH], F32, tag="bupsb")
                utf = up.tile([R, T], F16, tag="utf")

                vt_ps = [pvt.tile([R, 512], F32, tag=f"vt{i}", name=f"vt{i}")
                         for i in range(NT)]

                # ---- prep: scales, lora mats, biases (f32 staging tiles) ----
                stx = sp.tile([P, D], F32, tag="st32")
                v = stx[:, : 16 * 128].rearrange("p (o b) -> p o b", b=H // 64)
                nc.sync.dma_start(v, sdn.rearrange("(o p) b -> p o b", p=P))
                nc.vector.tensor_copy(sdnf[:], v)

                stx = sp.tile([P, D], F32, tag="st32")
                v = stx[:, : KH * R].rearrange("p (o r) -> p o r", r=R)
                nc.sync.dma_start(v, a2.rearrange("(o p) r -> p o r", p=P))
                nc.vector.tensor_copy(a2f[:], v)

                stx = sp.tile([P, D], F32, tag="st32")
                nc.sync.dma_start(stx[:R, :], b2)
                nc.sync.dma_start(stx[R:R + 1, :], bdn[None, :])
                nc.vector.tensor_copy(b2p[:], stx[:R + 1, :])

                stx = sp.tile([P, D], F32, tag="st32")
                v = stx[:].rearrange("p (o b) -> p o b", b=D // 64)
                nc.sync.dma_start(v, sup.rearrange("(o p) b -> p o b", p=P))
                nc.vector.tensor_copy(supf[:], v)

                stx = sp.tile([P, D], F32, tag="st32")
                v = stx[:, : KD * R].rearrange("p (o r) -> p o r", r=R)
                nc.sync.dma_start(v, a1.rearrange("(o p) r -> p o r", p=P))
                nc.vector.tensor_copy(a1f[:], v)

                for c in range(4):
                    stx = sp.tile([P, D], F32, tag="st32")
                    nc.sync.dma_start(stx[:R, :], b1[:, ts(c, D)])
                    nc.vector.tensor_copy(b1f[:, ts(c, D)], stx[:R, :])
                nc.sync.dma_start(bupsb[:], bup.rearrange("(o p) -> p o", p=P))

                # ---- x1 -> fp16, PE-transposed to [d_part, d_sub, token] ----
                for s in range(T // P):
                    stx = sp.tile([P, D], F32, tag="st32")
                    nc.sync.dma_start(stx[:], x1c[ts(s, P), :])
                    xf = sp.tile([P, D], F16, tag="qf")
                    nc.vector.tensor_copy(xf[:], stx[:])
                    for g in range(2):
                        tp = ptr.tile([P, 1024], F16, tag="tp")
                        for u in range(8):
                            nc.tensor.transpose(tp[:, ts(u, P)],
                                                xf[:, ts(8 * g + u, P)], ident[:])
                        nc.scalar.copy(
                            x1t[:, ds(8 * g, 8), ts(s, P)],
                            tp[:].rearrange("p (u t) -> p u t", t=P))

                # ---- uT = (x1 @ A1)^T : [R, T] ----
                for tt in range(NT):
                    ups = pp.tile([P, 512], F32, tag="mm")
                    for j in range(KD):
                        nc.tensor.matmul(ups[:R, :], a1f[:, j, :],
                                         x1t[:, j, ts(tt, 512)],
                                         start=(j == 0), stop=(j == KD - 1))
                    nc.scalar.copy(utf[:, ts(tt, 512)], ups[:R, :])

                # ---- UP: one 128-row slab of H per step ----
                for k in range(KH):
                    qst = sp.tile([P, D], I32, tag="qst")
                    nc.sync.dma_start(qst[:], wupq[ts(k, P), :])
                    qf = sp.tile([P, D], F16, tag="qf")
                    nc.vector.tensor_scalar_add(qf[:], qst[:], -7.5)
                    nc.vector.tensor_tensor(
                        qf[:].rearrange("p (b i) -> p b i", i=64),
                        qf[:].rearrange("p (b i) -> p b i", i=64),
                        supf[:, k, :, None].to_broadcast((P, D // 64, 64)),
                        mybir.AluOpType.mult)
                    wt = wp.tile([P, KD, P], F16, tag="wupt")
                    for g in range(2):
                        tp = ptr.tile([P, 1024], F16, tag="tp")
                        for u in range(8):
                            nc.tensor.transpose(tp[:, ts(u, P)],
                                                qf[:, ts(8 * g + u, P)], ident[:])
                        nc.scalar.copy(wt[:, ds(8 * g, 8), :],
                                       tp[:].rearrange("p (u t) -> p u t", t=P))

                    x2sl = xp.tile([P, T], F16, tag="x2sl")
                    for tt in range(NT):
                        ps = pp.tile([P, 512], F32, tag="mm")
                        for j in range(KD):
                            nc.tensor.matmul(ps[:], wt[:, j, :],
                                             x1t[:, j, ts(tt, 512)],
                                             start=(j == 0), stop=False)
                        nc.tensor.matmul(ps[:], b1f[:, ts(k, P)],
                                         utf[:, ts(tt, 512)],
                                         start=False, stop=True)
                        nc.scalar.activation(x2sl[:, ts(tt, 512)], ps[:],
                                             mybir.ActivationFunctionType.Relu,
                                             bias=bupsb[:, k:k + 1], scale=1.0)
                        nc.tensor.matmul(vt_ps[tt][:], a2f[:, k, :],
                                         x2sl[:, ts(tt, 512)],
                                         start=(k == 0), stop=(k == KH - 1),
                                         skip_group_check=True)
                    nc.sync.dma_start(x2td[k], x2sl[:])

                for tt in range(NT):
                    nc.scalar.copy(v1t[:R, ts(tt, 512)], vt_ps[tt][:])

            # ---- DOWN: x2 fully SBUF-resident; weights stream once ----
            with tc.tile_pool(name="x2res", bufs=1) as xr, \
                 tc.tile_pool(name="dstage", bufs=3) as dsp, \
                 tc.tile_pool(name="wdn", bufs=2) as wd, \
                 tc.tile_pool(name="yout", bufs=3) as yp:

                x2r = xr.tile([P, KH, T], F16, tag="x2r")
                for k in range(KH):
                    nc.sync.dma_start(x2r[:, k, :], x2td[k])

                for s in range(D // P):          # 16 d-slabs of 128
                    wdt = wd.tile([P, KH, P], F16, tag="wdt")
                    for c in range(8):           # 1024-wide h chunks
                        qst = dsp.tile([P, 1024], I32, tag="qd")
                        nc.sync.dma_start(qst[:], wdnq[ts(s, P), ts(c, 1024)])
                        qfd = dsp.tile([P, 1024], F16, tag="qfd")
                        nc.vector.tensor_scalar_add(qfd[:], qst[:], -7.5)
                        nc.vector.tensor_tensor(
                            qfd[:].rearrange("p (b i) -> p b i", i=64),
                            qfd[:].rearrange("p (b i) -> p b i", i=64),
                            sdnf[:, s, ds(16 * c, 16), None].to_broadcast(
                                (P, 16, 64)),
                            mybir.AluOpType.mult)
                        tp = ptr.tile([P, 1024], F16, tag="tp")
                        for u in range(8):
                            nc.tensor.transpose(tp[:, ts(u, P)],
                                                qfd[:, ts(u, P)], ident[:])
                        nc.scalar.copy(wdt[:, ds(8 * c, 8), :],
                                       tp[:].rearrange("p (u t) -> p u t", t=P))

                    for t8 in range(T // P):
                        ps = pp.tile([P, 512], F32, tag="mm")
                        for kk in range(KH):
                            nc.tensor.matmul(ps[:, :P], x2r[:, kk, ts(t8, P)],
                                             wdt[:, kk, :],
                                             start=(kk == 0), stop=False)
                        nc.tensor.matmul(ps[:, :P], v1t[:, ts(t8, P)],
                                         b2p[:, ts(s, P)],
                                         start=False, stop=True)
                        yo = yp.tile([P, P], F32, tag="yo")
                        nc.scalar.copy(yo[:], ps[:, :P])
                        nc.sync.dma_start(y2c[ts(t8, P), ts(s, P)], yo[:])

    nc.compile()
    return nc


_NC = None


def kernel(x1, w_up_q, w_up_scale, b_up, w_up_lora_a, w_up_lora_b,
           w_down_q, w_down_scale, b_down, w_down_lora_a, w_down_lora_b):
    global _NC, LAST_RESULTS
    if _NC is None:
        _NC = _build()

    x1 = np.ascontiguousarray(np.asarray(x1, dtype=np.float32))
    B, S, _ = x1.shape
    xf = x1.reshape(B * S, D)
    shared = {
        "wupq": np.ascontiguousarray(np.asarray(w_up_q, dtype=np.int32)),
        "sup": np.ascontiguousarray(np.asarray(w_up_scale, dtype=np.float32)),
        "bup": np.ascontiguousarray(np.asarray(b_up, dtype=np.float32)),
        "a1": np.ascontiguousarray(np.asarray(w_up_lora_a, dtype=np.float32)),
        "b1": np.ascontiguousarray(np.asarray(w_up_lora_b, dtype=np.float32)),
        "wdnq": np.ascontiguousarray(np.asarray(w_down_q, dtype=np.int32)),
        "sdn": np.ascontiguousarray(np.asarray(w_down_scale, dtype=np.float32)),
        "bdn": np.ascontiguousarray(np.asarray(b_down, dtype=np.float32)),
        "a2": np.ascontiguousarray(np.asarray(w_down_lora_a, dtype=np.float32)),
        "b2": np.ascontiguousarray(np.asarray(w_down_lora_b, dtype=np.float32)),
    }
    in_maps = [{"x1c": np.ascontiguousarray(xf[c * T:(c + 1) * T]), **shared}
               for c in range(NCORES)]

    res = run_bass_kernel_spmd(_NC, in_maps, core_ids=list(range(NCORES)),
                               trace=TRACE)
    LAST_RESULTS = res
    out = np.concatenate([res.results[c]["y2c"] for c in range(NCORES)], axis=0)
    return out.reshape(B, S, D)


# revision 8
# speedup vs baseline: 1.0200x; 1.0060x over previous
"""Trainium2 Bass kernel for quantized-MLP-with-LoRA (nn_MixedSparseTraditionalMLP).

Strategy: data-parallel over the 8192 tokens across 8 NeuronCores (1024 tokens
per core). Each core holds the full weights, dequantizes the 4-bit codes to
fp16 on-chip (DVE add + broadcast-scale multiply), and transposes operands into
contraction-major layout with the TENSOR ENGINE (matmul transpose mode, 1
cycle/row for fp16) instead of the DMA xbar, whose per-element descriptors
dominated the original version. Both projections run as fp16 matmuls with fp32
PSUM accumulation. The hidden activation x2 [8192h x 1024t] stays fully
SBUF-resident in fp16: the up projection's relu writes straight into it and
the down projection reads its [128,128] chunks as stationary matmul operands,
so x2 never touches DRAM. Weight-slab builds (dequant + PE transpose + copy)
are software-pipelined one slab ahead of the matmuls that consume them, and
the low-rank (LoRA) PSUM accumulation steps are deferred one slab so the PE
never waits on the scalar engine. Small constants (scales, LoRA mats, biases)
are pre-cast to fp16 and pre-laid-out on the host -- numerically identical to
the on-device casts they replace; the quantized weight matrices stream as
int32 exactly as given. No collectives: the host concatenates the 8 per-core
token slices.
"""
import sys

if "/opt/trn_rl_repo" not in sys.path:
    sys.path.insert(0, "/opt/trn_rl_repo")

import numpy as np

import concourse.bass as bass
import concourse.mybir as mybir
import concourse.tile as tile
from concourse import bacc
from concourse.bass import ts, ds
from concourse.bass_utils import run_bass_kernel_spmd
from concourse.masks import make_identity

F16 = mybir.dt.float16
F32 = mybir.dt.float32
I32 = mybir.dt.int32

NCORES = 8
T = 1024          # tokens per core
D = 2048
H = 8192
R = 16
P = 128
KD = D // P       # 16 k-subtiles for the up contraction
KH = H // P       # 64 k-subtiles for the down contraction
NT = T // 512     # 2 moving-operand tiles of 512 tokens

TRACE = False
LAST_RESULTS = None


def _build():
    nc = bacc.Bacc("TRN2", target_bir_lowering=False, debug=False,
                   enable_asserts=False, num_devices=NCORES)

    x1c = nc.dram_tensor("x1c", [T, D], F16, kind="ExternalInput").ap()
    wupq = nc.dram_tensor("wupq", [H, D], I32, kind="ExternalInput").ap()
    supin = nc.dram_tensor("supin", [P, KH, D // 64], F16,
                           kind="ExternalInput").ap()
    bupin = nc.dram_tensor("bupin", [P, KH], F32, kind="ExternalInput").ap()
    a1in = nc.dram_tensor("a1in", [P, KD, R], F16, kind="ExternalInput").ap()
    b1in = nc.dram_tensor("b1in", [R, H], F16, kind="ExternalInput").ap()
    wdnq = nc.dram_tensor("wdnq", [D, H], I32, kind="ExternalInput").ap()
    sdnin = nc.dram_tensor("sdnin", [P, KD, H // 64], F16,
                           kind="ExternalInput").ap()
    a2in = nc.dram_tensor("a2in", [P, KH, R], F16, kind="ExternalInput").ap()
    b2pin = nc.dram_tensor("b2pin", [R + 1, D], F16, kind="ExternalInput").ap()
    y2c = nc.dram_tensor("y2c", [T, D], F32, kind="ExternalOutput").ap()

    with tile.TileContext(nc) as tc:
        with tc.tile_pool(name="gconst", bufs=1) as gp, \
             tc.tile_pool(name="x2res", bufs=1) as xr, \
             tc.tile_pool(name="psum_mm", bufs=3, space="PSUM") as pp, \
             tc.tile_pool(name="psum_tr", bufs=2, space="PSUM") as ptr:

            ident = gp.tile([P, P], F16, tag="ident")
            make_identity(nc, ident[:])
            a2f = gp.tile([P, KH, R], F16, tag="a2f")
            v1t = gp.tile([R + 1, T], F16, tag="v1t")
            # row R must read 1.0 (folds b_down into the lora matmul); rows
            # 0..R-1 are overwritten with vT after the up phase
            nc.any.memset(v1t[:], 1.0)

            x2r = xr.tile([P, KH, T], F16, tag="x2r")   # resident hidden act

            with tc.tile_pool(name="upc", bufs=1) as up, \
                 tc.tile_pool(name="stage", bufs=3) as sp, \
                 tc.tile_pool(name="b1sp", bufs=3) as bsp, \
                 tc.tile_pool(name="wup", bufs=2) as wp, \
                 tc.tile_pool(name="psum_vt", bufs=1, space="PSUM") as pvt:

                x1t = up.tile([P, KD, T], F16, tag="x1t")
                supf = up.tile([P, KH, D // 64], F16, tag="supf")
                a1f = up.tile([P, KD, R], F16, tag="a1f")
                bupsb = up.tile([P, KH], F32, tag="bupsb")
                utf = up.tile([R, T], F16, tag="utf")

                vt_ps = [pvt.tile([R, 512], F32, tag=f"vt{i}", name=f"vt{i}")
                         for i in range(NT)]

                def up_chunk(k, c):
                    qst = sp.tile([P, 1024], I32, tag="qst")
                    nc.sync.dma_start(qst[:], wupq[ts(k, P), ts(c, 1024)])
                    qf = sp.tile([P, 1024], F16, tag="qf")
                    nc.vector.tensor_scalar_add(qf[:], qst[:], -7.5)
                    nc.vector.tensor_tensor(
                        qf[:].rearrange("p (b i) -> p b i", i=64),
                        qf[:].rearrange("p (b i) -> p b i", i=64),
                        supf[:, k, ds(16 * c, 16), None].to_broadcast(
                            (P, 16, 64)),
                        mybir.AluOpType.mult)
                    return qf

                # ---- prep: scale first (dequant needs it), then k=0 chunks,
                # x1 chunks (straight f16 loads -> PE transpose), consts ----
                nc.sync.dma_start(supf[:], supin)
                pending0 = [up_chunk(0, c) for c in range(2)]
                nc.sync.dma_start(bupsb[:], bupin)
                for s in range(T // P):
                    for g in range(2):
                        xf = sp.tile([P, 1024], F16, tag="xf")
                        nc.sync.dma_start(xf[:], x1c[ts(s, P), ts(g, 1024)])
                        tp = ptr.tile([P, 1024], F16, tag="tp")
                        for u in range(8):
                            nc.tensor.transpose(tp[:, ts(u, P)],
                                                xf[:, ts(u, P)], ident[:])
                        nc.scalar.copy(
                            x1t[:, ds(8 * g, 8), ts(s, P)],
                            tp[:].rearrange("p (u t) -> p u t", t=P))
                nc.sync.dma_start(a1f[:], a1in)
                nc.sync.dma_start(a2f[:], a2in)

                def build_wt(k, qfs=None):
                    b1s = bsp.tile([R, P], F16, tag="b1s")
                    nc.sync.dma_start(b1s[:], b1in[:, ts(k, P)])
                    if qfs is None:
                        qfs = [up_chunk(k, c) for c in range(2)]
                    wt = wp.tile([P, KD, P], F16, tag="wupt")
                    for c in range(2):
                        tp = ptr.tile([P, 1024], F16, tag="tp")
                        for u in range(8):
                            nc.tensor.transpose(tp[:, ts(u, P)],
                                                qfs[c][:, ts(u, P)], ident[:])
                        nc.scalar.copy(wt[:, ds(8 * c, 8), :],
                                       tp[:].rearrange("p (u t) -> p u t", t=P))
                    return wt, b1s

                # ---- UP: one 128-row slab of H per step; weight builds are
                # pipelined one slab ahead; vt lora matmuls lag one slab ----
                cur = build_wt(0, pending0)
                for k in range(KH):
                    nxt = build_wt(k + 1) if k + 1 < KH else None
                    if k > 0:
                        for tt in range(NT):
                            nc.tensor.matmul(vt_ps[tt][:], a2f[:, k - 1, :],
                                             x2r[:, k - 1, ts(tt, 512)],
                                             start=(k - 1 == 0), stop=False,
                                             skip_group_check=True)
                    wt, b1s = cur
                    ps_l = []
                    for tt in range(NT):
                        ps = pp.tile([P, 512], F32, tag="mm")
                        for j in range(KD):
                            nc.tensor.matmul(ps[:], wt[:, j, :],
                                             x1t[:, j, ts(tt, 512)],
                                             start=(j == 0), stop=False)
                        ps_l.append(ps)
                    if k == 0:
                        # uT = (x1 @ A1)^T, interleaved inside the open groups
                        for tt in range(NT):
                            ut = pvt.tile([R, 512], F32, tag="ut")
                            for j in range(KD):
                                nc.tensor.matmul(ut[:], a1f[:, j, :],
                                                 x1t[:, j, ts(tt, 512)],
                                                 start=(j == 0),
                                                 stop=(j == KD - 1),
                                                 skip_group_check=True)
                            nc.scalar.copy(utf[:, ts(tt, 512)], ut[:])
                    for tt in range(NT):
                        nc.tensor.matmul(ps_l[tt][:], b1s[:],
                                         utf[:, ts(tt, 512)],
                                         start=False, stop=True)
                        nc.scalar.activation(x2r[:, k, ts(tt, 512)], ps_l[tt][:],
                                             mybir.ActivationFunctionType.Relu,
                                             bias=bupsb[:, k:k + 1], scale=1.0)
                    cur = nxt

                for tt in range(NT):
                    nc.tensor.matmul(vt_ps[tt][:], a2f[:, KH - 1, :],
                                     x2r[:, KH - 1, ts(tt, 512)],
                                     start=False, stop=True,
                                     skip_group_check=True)
                    nc.scalar.copy(v1t[:R, ts(tt, 512)], vt_ps[tt][:])

            # ---- DOWN: x2 already resident; weights stream once,
            # builds pipelined one d-slab ahead ----
            with tc.tile_pool(name="dconst", bufs=1) as dc, \
                 tc.tile_pool(name="dstage", bufs=3) as dsp, \
                 tc.tile_pool(name="wdn", bufs=2) as wd, \
                 tc.tile_pool(name="yout", bufs=3) as yp:

                sdnf = dc.tile([P, KD, H // 64], F16, tag="sdnf")
                b2p = dc.tile([R + 1, D], F16, tag="b2p")
                nc.sync.dma_start(sdnf[:], sdnin)
                nc.sync.dma_start(b2p[:], b2pin)

                def build_wdt(s):
                    wdt = wd.tile([P, KH, P], F16, tag="wdt")
                    for c in range(8):           # 1024-wide h chunks
                        qst = dsp.tile([P, 1024], I32, tag="qd")
                        nc.sync.dma_start(qst[:], wdnq[ts(s, P), ts(c, 1024)])
                        qfd = dsp.tile([P, 1024], F16, tag="qfd")
                        nc.vector.tensor_scalar_add(qfd[:], qst[:], -7.5)
                        nc.vector.tensor_tensor(
                            qfd[:].rearrange("p (b i) -> p b i", i=64),
                            qfd[:].rearrange("p (b i) -> p b i", i=64),
                            sdnf[:, s, ds(16 * c, 16), None].to_broadcast(
                                (P, 16, 64)),
                            mybir.AluOpType.mult)
                        tp = ptr.tile([P, 1024], F16, tag="tp")
                        for u in range(8):
                            nc.tensor.transpose(tp[:, ts(u, P)],
                                                qfd[:, ts(u, P)], ident[:])
                        nc.scalar.copy(wdt[:, ds(8 * c, 8), :],
                                       tp[:].rearrange("p (u t) -> p u t", t=P))
                    return wdt

                cur = build_wdt(0)
                for s in range(D // P):          # 16 d-slabs of 128
                    nxt = build_wdt(s + 1) if s + 1 < D // P else None
                    wdt = cur
                    for t8 in range(T // P):
                        ps = pp.tile([P, 512], F32, tag="mm")
                        for kk in range(KH):
                            nc.tensor.matmul(ps[:, :P], x2r[:, kk, ts(t8, P)],
                                             wdt[:, kk, :],
                                             start=(kk == 0), stop=False)
                        nc.tensor.matmul(ps[:, :P], v1t[:, ts(t8, P)],
                                         b2p[:, ts(s, P)],
                                         start=False, stop=True)
                        yo = yp.tile([P, P], F32, tag="yo")
                        nc.scalar.copy(yo[:], ps[:, :P])
                        nc.sync.dma_start(y2c[ts(t8, P), ts(s, P)], yo[:])
                    cur = nxt

    nc.compile()
    return nc


def make_in_maps(x1, w_up_q, w_up_scale, b_up, w_up_lora_a, w_up_lora_b,
                 w_down_q, w_down_scale, b_down, w_down_lora_a, w_down_lora_b):
    """Host-side prep: dtype casts + SBUF-layout permutes of the small
    constants; the quantized weights pass through as int32 exactly as given."""
    f16 = np.float16
    ca = np.ascontiguousarray
    xf = ca(np.asarray(x1, np.float32).reshape(-1, D).astype(f16))
    shared = {
        "wupq": ca(np.asarray(w_up_q, np.int32)),
        "supin": ca(np.asarray(w_up_scale, np.float32).astype(f16)
                    .reshape(KH, P, D // 64).transpose(1, 0, 2)),
        "bupin": ca(np.asarray(b_up, np.float32).reshape(KH, P).T),
        "a1in": ca(np.asarray(w_up_lora_a, np.float32).astype(f16)
                   .reshape(KD, P, R).transpose(1, 0, 2)),
        "b1in": ca(np.asarray(w_up_lora_b, np.float32).astype(f16)),
        "wdnq": ca(np.asarray(w_down_q, np.int32)),
        "sdnin": ca(np.asarray(w_down_scale, np.float32).astype(f16)
                    .reshape(KD, P, H // 64).transpose(1, 0, 2)),
        "a2in": ca(np.asarray(w_down_lora_a, np.float32).astype(f16)
                   .reshape(KH, P, R).transpose(1, 0, 2)),
        "b2pin": ca(np.concatenate(
            [np.asarray(w_down_lora_b, np.float32),
             np.asarray(b_down, np.float32)[None, :]], 0).astype(f16)),
    }
    return [{"x1c": ca(xf[c * T:(c + 1) * T]), **shared}
            for c in range(NCORES)]


_NC = None


def kernel(x1, w_up_q, w_up_scale, b_up, w_up_lora_a, w_up_lora_b,
           w_down_q, w_down_scale, b_down, w_down_lora_a, w_down_lora_b):
    global _NC, LAST_RESULTS
    if _NC is None:
        _NC = _build()

    x1 = np.asarray(x1)
    B, S, _ = x1.shape
    in_maps = make_in_maps(x1, w_up_q, w_up_scale, b_up, w_up_lora_a,
                           w_up_lora_b, w_down_q, w_down_scale, b_down,
                           w_down_lora_a, w_down_lora_b)

    res = run_bass_kernel_spmd(_NC, in_maps, core_ids=list(range(NCORES)),
                               trace=TRACE)
    LAST_RESULTS = res
    out = np.concatenate([res.results[c]["y2c"] for c in range(NCORES)], axis=0)
    return out.reshape(B, S, D)


# revision 9
# speedup vs baseline: 1.0259x; 1.0058x over previous
"""Trainium2 Bass kernel for quantized-MLP-with-LoRA (nn_MixedSparseTraditionalMLP).

Strategy: data-parallel over the 8192 tokens across 8 NeuronCores (1024 tokens
per core). Each core holds the full weights, dequantizes the 4-bit codes to
fp16 on-chip (DVE add + broadcast-scale multiply), and transposes operands into
contraction-major layout with the TENSOR ENGINE (matmul transpose mode, 1
cycle/row for fp16) instead of the DMA xbar, whose per-element descriptors
dominated the original version. Both projections run as fp16 matmuls with fp32
PSUM accumulation. The hidden activation x2 [8192h x 1024t] stays fully
SBUF-resident in fp16: the up projection's relu writes straight into it and
the down projection reads its [128,128] chunks as stationary matmul operands,
so x2 never touches DRAM. Weight-slab builds (dequant + PE transpose + copy)
are software-pipelined one slab ahead of the matmuls that consume them, and
the low-rank (LoRA) PSUM accumulation steps are deferred one slab so the PE
never waits on the scalar engine. Small constants (scales, LoRA mats, biases)
are pre-cast to fp16 and pre-laid-out on the host -- numerically identical to
the on-device casts they replace; the quantized weight matrices stream as
int32 exactly as given. No collectives: the host concatenates the 8 per-core
token slices.
"""
import sys

if "/opt/trn_rl_repo" not in sys.path:
    sys.path.insert(0, "/opt/trn_rl_repo")

import numpy as np

import concourse.bass as bass
import concourse.mybir as mybir
import concourse.tile as tile
from concourse import bacc
from concourse.bass import ts, ds
from concourse.bass_utils import run_bass_kernel_spmd
from concourse.masks import make_identity

F16 = mybir.dt.float16
F32 = mybir.dt.float32
I32 = mybir.dt.int32
U8 = mybir.dt.uint8

NCORES = 8
T = 1024          # tokens per core
D = 2048
H = 8192
R = 16
P = 128
KD = D // P       # 16 k-subtiles for the up contraction
KH = H // P       # 64 k-subtiles for the down contraction
NT = T // 512     # 2 moving-operand tiles of 512 tokens

TRACE = False
LAST_RESULTS = None


def _build():
    nc = bacc.Bacc("TRN2", target_bir_lowering=False, debug=False,
                   enable_asserts=False, num_devices=NCORES)

    x1c = nc.dram_tensor("x1c", [T, D], F16, kind="ExternalInput").ap()
    wupq = nc.dram_tensor("wupq", [H, D], U8, kind="ExternalInput").ap()
    supin = nc.dram_tensor("supin", [P, KH, D // 64], F16,
                           kind="ExternalInput").ap()
    bupin = nc.dram_tensor("bupin", [P, KH], F32, kind="ExternalInput").ap()
    a1in = nc.dram_tensor("a1in", [P, KD, R], F16, kind="ExternalInput").ap()
    b1in = nc.dram_tensor("b1in", [R, H], F16, kind="ExternalInput").ap()
    wdnq = nc.dram_tensor("wdnq", [D, H], U8, kind="ExternalInput").ap()
    sdnin = nc.dram_tensor("sdnin", [P, KD, H // 64], F16,
                           kind="ExternalInput").ap()
    a2in = nc.dram_tensor("a2in", [P, KH, R], F16, kind="ExternalInput").ap()
    b2pin = nc.dram_tensor("b2pin", [R + 1, D], F16, kind="ExternalInput").ap()
    y2c = nc.dram_tensor("y2c", [T, D], F16, kind="ExternalOutput").ap()

    with tile.TileContext(nc) as tc:
        with tc.tile_pool(name="gconst", bufs=1) as gp, \
             tc.tile_pool(name="x2res", bufs=1) as xr, \
             tc.tile_pool(name="psum_mm", bufs=3, space="PSUM") as pp, \
             tc.tile_pool(name="psum_tr", bufs=2, space="PSUM") as ptr:

            ident = gp.tile([P, P], F16, tag="ident")
            make_identity(nc, ident[:])
            a2f = gp.tile([P, KH, R], F16, tag="a2f")
            v1t = gp.tile([R + 1, T], F16, tag="v1t")
            # row R must read 1.0 (folds b_down into the lora matmul); rows
            # 0..R-1 are overwritten with vT after the up phase
            nc.any.memset(v1t[:], 1.0)

            x2r = xr.tile([P, KH, T], F16, tag="x2r")   # resident hidden act

            with tc.tile_pool(name="upc", bufs=1) as up, \
                 tc.tile_pool(name="stage", bufs=3) as sp, \
                 tc.tile_pool(name="b1sp", bufs=3) as bsp, \
                 tc.tile_pool(name="wup", bufs=2) as wp, \
                 tc.tile_pool(name="psum_vt", bufs=1, space="PSUM") as pvt:

                x1t = up.tile([P, KD, T], F16, tag="x1t")
                supf = up.tile([P, KH, D // 64], F16, tag="supf")
                a1f = up.tile([P, KD, R], F16, tag="a1f")
                bupsb = up.tile([P, KH], F32, tag="bupsb")
                utf = up.tile([R, T], F16, tag="utf")

                vt_ps = [pvt.tile([R, 512], F32, tag=f"vt{i}", name=f"vt{i}")
                         for i in range(NT)]

                def up_chunk(k, c):
                    qst = sp.tile([P, 1024], U8, tag="qst")
                    nc.sync.dma_start(qst[:], wupq[ts(k, P), ts(c, 1024)])
                    qf = sp.tile([P, 1024], F16, tag="qf")
                    nc.vector.tensor_scalar_add(qf[:], qst[:], -7.5)
                    nc.vector.tensor_tensor(
                        qf[:].rearrange("p (b i) -> p b i", i=64),
                        qf[:].rearrange("p (b i) -> p b i", i=64),
                        supf[:, k, ds(16 * c, 16), None].to_broadcast(
                            (P, 16, 64)),
                        mybir.AluOpType.mult)
                    return qf

                # ---- prep: scale first (dequant needs it), then k=0 chunks,
                # x1 chunks (straight f16 loads -> PE transpose), consts ----
                def x1_slab(s):
                    for g in range(2):
                        xf = sp.tile([P, 1024], F16, tag="xf")
                        nc.sync.dma_start(xf[:], x1c[ts(s, P), ts(g, 1024)])
                        tp = ptr.tile([P, 1024], F16, tag="tp")
                        for u in range(8):
                            nc.tensor.transpose(tp[:, ts(u, P)],
                                                xf[:, ts(u, P)], ident[:])
                        nc.scalar.copy(
                            x1t[:, ds(8 * g, 8), ts(s, P)],
                            tp[:].rearrange("p (u t) -> p u t", t=P))

                x1_slab(0)
                nc.sync.dma_start(supf[:], supin)
                x1_slab(1)
                pending0 = [up_chunk(0, c) for c in range(2)]
                nc.sync.dma_start(bupsb[:], bupin)
                for s in range(2, T // P):
                    for g in range(2):
                        xf = sp.tile([P, 1024], F16, tag="xf")
                        nc.sync.dma_start(xf[:], x1c[ts(s, P), ts(g, 1024)])
                        tp = ptr.tile([P, 1024], F16, tag="tp")
                        for u in range(8):
                            nc.tensor.transpose(tp[:, ts(u, P)],
                                                xf[:, ts(u, P)], ident[:])
                        nc.scalar.copy(
                            x1t[:, ds(8 * g, 8), ts(s, P)],
                            tp[:].rearrange("p (u t) -> p u t", t=P))
                nc.sync.dma_start(a1f[:], a1in)
                nc.sync.dma_start(a2f[:], a2in)

                def build_wt(k, qfs=None):
                    b1s = bsp.tile([R, P], F16, tag="b1s")
                    nc.sync.dma_start(b1s[:], b1in[:, ts(k, P)])
                    if qfs is None:
                        qfs = [up_chunk(k, c) for c in range(2)]
                    wt = wp.tile([P, KD, P], F16, tag="wupt")
                    for c in range(2):
                        tp = ptr.tile([P, 1024], F16, tag="tp")
                        for u in range(8):
                            nc.tensor.transpose(tp[:, ts(u, P)],
                                                qfs[c][:, ts(u, P)], ident[:])
                        nc.scalar.copy(wt[:, ds(8 * c, 8), :],
                                       tp[:].rearrange("p (u t) -> p u t", t=P))
                    return wt, b1s

                # ---- UP: one 128-row slab of H per step; weight builds are
                # pipelined one slab ahead; vt lora matmuls lag one slab ----
                cur = build_wt(0, pending0)
                for k in range(KH):
                    nxt = build_wt(k + 1) if k + 1 < KH else None
                    if k > 0:
                        for tt in range(NT):
                            nc.tensor.matmul(vt_ps[tt][:], a2f[:, k - 1, :],
                                             x2r[:, k - 1, ts(tt, 512)],
                                             start=(k - 1 == 0), stop=False,
                                             skip_group_check=True)
                    wt, b1s = cur
                    ps_l = []
                    for tt in range(NT):
                        ps = pp.tile([P, 512], F32, tag="mm")
                        for j in range(KD):
                            nc.tensor.matmul(ps[:], wt[:, j, :],
                                             x1t[:, j, ts(tt, 512)],
                                             start=(j == 0), stop=False)
                        ps_l.append(ps)
                    if k == 0:
                        # uT = (x1 @ A1)^T, interleaved inside the open groups
                        for tt in range(NT):
                            ut = pvt.tile([R, 512], F32, tag="ut")
                            for j in range(KD):
                                nc.tensor.matmul(ut[:], a1f[:, j, :],
                                                 x1t[:, j, ts(tt, 512)],
                                                 start=(j == 0),
                                                 stop=(j == KD - 1),
                                                 skip_group_check=True)
                            nc.scalar.copy(utf[:, ts(tt, 512)], ut[:])
                    for tt in range(NT):
                        nc.tensor.matmul(ps_l[tt][:], b1s[:],
                                         utf[:, ts(tt, 512)],
                                         start=False, stop=True)
                        nc.scalar.activation(x2r[:, k, ts(tt, 512)], ps_l[tt][:],
                                             mybir.ActivationFunctionType.Relu,
                                             bias=bupsb[:, k:k + 1], scale=1.0)
                    cur = nxt

                for tt in range(NT):
                    nc.tensor.matmul(vt_ps[tt][:], a2f[:, KH - 1, :],
                                     x2r[:, KH - 1, ts(tt, 512)],
                                     start=False, stop=True,
                                     skip_group_check=True)
                    nc.scalar.copy(v1t[:R, ts(tt, 512)], vt_ps[tt][:])

            # ---- DOWN: x2 already resident; weights stream once,
            # builds pipelined one d-slab ahead ----
            with tc.tile_pool(name="dconst", bufs=1) as dc, \
                 tc.tile_pool(name="dstage", bufs=3) as dsp, \
                 tc.tile_pool(name="wdn", bufs=2) as wd, \
                 tc.tile_pool(name="yout", bufs=3) as yp:

                sdnf = dc.tile([P, KD, H // 64], F16, tag="sdnf")
                b2p = dc.tile([R + 1, D], F16, tag="b2p")
                nc.sync.dma_start(sdnf[:], sdnin)
                nc.sync.dma_start(b2p[:], b2pin)

                def build_wdt(s):
                    wdt = wd.tile([P, KH, P], F16, tag="wdt")
                    for c in range(8):           # 1024-wide h chunks
                        qst = dsp.tile([P, 1024], U8, tag="qd")
                        nc.sync.dma_start(qst[:], wdnq[ts(s, P), ts(c, 1024)])
                        qfd = dsp.tile([P, 1024], F16, tag="qfd")
                        nc.vector.tensor_scalar_add(qfd[:], qst[:], -7.5)
                        nc.vector.tensor_tensor(
                            qfd[:].rearrange("p (b i) -> p b i", i=64),
                            qfd[:].rearrange("p (b i) -> p b i", i=64),
                            sdnf[:, s, ds(16 * c, 16), None].to_broadcast(
                                (P, 16, 64)),
                            mybir.AluOpType.mult)
                        tp = ptr.tile([P, 1024], F16, tag="tp")
                        for u in range(8):
                            nc.tensor.transpose(tp[:, ts(u, P)],
                                                qfd[:, ts(u, P)], ident[:])
                        nc.scalar.copy(wdt[:, ds(8 * c, 8), :],
                                       tp[:].rearrange("p (u t) -> p u t", t=P))
                    return wdt

                cur = build_wdt(0)
                for s in range(D // P):          # 16 d-slabs of 128
                    nxt = build_wdt(s + 1) if s + 1 < D // P else None
                    wdt = cur
                    for t8 in range(T // P):
                        ps = pp.tile([P, 512], F32, tag="mm")
                        for kk in range(KH):
                            nc.tensor.matmul(ps[:, :P], x2r[:, kk, ts(t8, P)],
                                             wdt[:, kk, :],
                                             start=(kk == 0), stop=False)
                        nc.tensor.matmul(ps[:, :P], v1t[:, ts(t8, P)],
                                         b2p[:, ts(s, P)],
                                         start=False, stop=True)
                        yo = yp.tile([P, P], F16, tag="yo")
                        nc.scalar.copy(yo[:], ps[:, :P])
                        nc.sync.dma_start(y2c[ts(t8, P), ts(s, P)], yo[:])
                    cur = nxt

    nc.compile()
    return nc


def make_in_maps(x1, w_up_q, w_up_scale, b_up, w_up_lora_a, w_up_lora_b,
                 w_down_q, w_down_scale, b_down, w_down_lora_a, w_down_lora_b):
    """Host-side prep: dtype casts + SBUF-layout permutes of the small
    constants; the quantized weights pass through as int32 exactly as given."""
    f16 = np.float16
    ca = np.ascontiguousarray
    xf = ca(np.asarray(x1, np.float32).reshape(-1, D).astype(f16))
    shared = {
        "wupq": ca(np.asarray(w_up_q).astype(np.uint8)),
        "supin": ca(np.asarray(w_up_scale, np.float32).astype(f16)
                    .reshape(KH, P, D // 64).transpose(1, 0, 2)),
        "bupin": ca(np.asarray(b_up, np.float32).reshape(KH, P).T),
        "a1in": ca(np.asarray(w_up_lora_a, np.float32).astype(f16)
                   .reshape(KD, P, R).transpose(1, 0, 2)),
        "b1in": ca(np.asarray(w_up_lora_b, np.float32).astype(f16)),
        "wdnq": ca(np.asarray(w_down_q).astype(np.uint8)),
        "sdnin": ca(np.asarray(w_down_scale, np.float32).astype(f16)
                    .reshape(KD, P, H // 64).transpose(1, 0, 2)),
        "a2in": ca(np.asarray(w_down_lora_a, np.float32).astype(f16)
                   .reshape(KH, P, R).transpose(1, 0, 2)),
        "b2pin": ca(np.concatenate(
            [np.asarray(w_down_lora_b, np.float32),
             np.asarray(b_down, np.float32)[None, :]], 0).astype(f16)),
    }
    return [{"x1c": ca(xf[c * T:(c + 1) * T]), **shared}
            for c in range(NCORES)]


_NC = None


def kernel(x1, w_up_q, w_up_scale, b_up, w_up_lora_a, w_up_lora_b,
           w_down_q, w_down_scale, b_down, w_down_lora_a, w_down_lora_b):
    global _NC, LAST_RESULTS
    if _NC is None:
        _NC = _build()

    x1 = np.asarray(x1)
    B, S, _ = x1.shape
    in_maps = make_in_maps(x1, w_up_q, w_up_scale, b_up, w_up_lora_a,
                           w_up_lora_b, w_down_q, w_down_scale, b_down,
                           w_down_lora_a, w_down_lora_b)

    res = run_bass_kernel_spmd(_NC, in_maps, core_ids=list(range(NCORES)),
                               trace=TRACE)
    LAST_RESULTS = res
    out = np.concatenate([res.results[c]["y2c"] for c in range(NCORES)], axis=0)
    return out.reshape(B, S, D).astype(np.float32)
